# revision 22
# baseline (speedup 1.0000x reference)
"""Trainium2 Bass kernel for nn_DecoderBlock (dynamic-conv decoder block).

v2: predictor weights sharded over cores by output column + on-chip AllToAll;
all matmul streams in bf16; weights SBUF-resident; ~75 DMAs total.

Sharding: data-parallel over batch for the conv phases (2 samples/core);
the kernel-predictor matmul (kpsw: [4609, 4096], the dominant HBM traffic)
is column-sharded 8 ways — each core computes dw/pw/bias columns for ALL 16
samples, then one AllToAll (bf16, ~330 KB) redistributes to sample owners.

Math per sample (C=512, G=64, cg=8, H=W=32, S=512, Cout=256):
  dw   = conv3x3(reflect_pad(w), kp_sw) + kp_sb        # kernel predictor
  pw   = pooled @ kp_pw.T + kp_pb ;  bias = pooled @ kp_bw.T + kp_bb
  xn   = instance_norm(x)
  y    = grouped_dynconv3x3(reflect_pad(xn), dw)       # per-sample weights
  y    = grouped_pointwise(pw, y) + bias
  y    = relu(conv3x3(y, dec_w1) + b1)
  y    = relu(conv3x3(y, dec_w2) + b2)
  out  = nearest_upsample_2x(y)

Grouped convs use block-diagonal [128,128] weight tiles built by scattering
the AllToAll output through a DRAM scratch with a stride-1032
diagonal-embedding view (only rearrange-able access patterns needed).
"""

import sys

sys.path.insert(0, "/opt/trn_rl_repo")

import numpy as np
import ml_dtypes

import concourse.bacc as bacc
import concourse.tile as tile
from concourse import mybir
from concourse.alu_op_type import AluOpType
from concourse.bass_utils import run_bass_kernel_spmd

F32 = mybir.dt.float32
BF16 = mybir.dt.bfloat16
AF = mybir.ActivationFunctionType

NCORES = 8
B = 16           # total batch
BPC = 2          # samples per core
C = 512          # in channels
CO = 256         # out channels
S = 512          # style dim
G = 64           # groups
CG = 8           # channels per group
H = W = 32
HW = H * W
NT = C // 128    # 4 channel tiles
NM2 = CO // 128  # 2 out-channel tiles
EPS = 1e-5
SCR = 128 * 129  # 16512; span of the diag-embedded identity build
CSL = C * CG // NCORES   # 512 predictor cols per core
NTAP = 10        # 9 dw taps + the pw tap, interleaved per predictor column
CCW = NTAP * CSL  # 5120 = per-sample slab width in the cc buffer
BP = B * 9       # 144 = (sample, out-position) columns in predictor matmul

_CACHE = {}


def _build():
    nc = bacc.Bacc(None, target_bir_lowering=False)

    x2 = nc.declare_dram_parameter("x2", [BPC, C, H, W], F32, isOutput=False)
    wall = nc.declare_dram_parameter("wall", [B, S, 3, 3], F32, isOutput=False)
    wown = nc.declare_dram_parameter("wown", [BPC, S, 3, 3], F32, isOutput=False)
    kpsws = nc.declare_dram_parameter("kpsws", [9 * S + 1, CSL], BF16, isOutput=False)
    kppws = nc.declare_dram_parameter("kppws", [S + 1, CSL], BF16, isOutput=False)
    kpbw = nc.declare_dram_parameter("kpbw", [S + 1, C], BF16, isOutput=False)
    w1t = nc.declare_dram_parameter("w1t", [9, C, C], BF16, isOutput=False)
    w2t = nc.declare_dram_parameter("w2t", [9, C, CO], BF16, isOutput=False)
    b1d = nc.declare_dram_parameter("b1d", [C], F32, isOutput=False)
    b2d = nc.declare_dram_parameter("b2d", [CO], F32, isOutput=False)
    yout = nc.declare_dram_parameter("yout", [BPC, CO, 2 * H, 2 * W], F32, isOutput=True)

    ccin = nc.dram_tensor("ccin", [B, CCW], BF16)
    ccout = nc.dram_tensor("ccout", [B, CCW], BF16)
    idscr = nc.dram_tensor("idscr", [SCR], BF16)

    with tile.TileContext(nc) as tc:
        with (
            tc.tile_pool(name="consts", bufs=1) as consts,
            tc.tile_pool(name="kstream", bufs=3) as kstream,
            tc.tile_pool(name="wres", bufs=1) as wres,
            tc.tile_pool(name="blk", bufs=2) as blkp,
            tc.tile_pool(name="act", bufs=2) as actp,
            tc.tile_pool(name="pad3", bufs=6) as pad3,
            tc.tile_pool(name="outp", bufs=2) as outp,
            tc.tile_pool(name="psum", bufs=6, space="PSUM") as psum,
        ):
            # ---------------- persistent small constants ----------------
            b1sb = consts.tile([128, NT], F32, tag="b1sb")
            nc.sync.dma_start(out=b1sb[:, :], in_=b1d.rearrange("(m c) -> c m", c=128))
            b2sb = consts.tile([128, NM2], F32, tag="b2sb")
            nc.sync.dma_start(out=b2sb[:, :], in_=b2d.rearrange("(m c) -> c m", c=128))

            epsb = consts.tile([128, 1], F32, tag="epsb")
            nc.vector.memset(epsb[:], EPS)
            onesf = consts.tile([1, BP], F32, tag="onesf")
            nc.vector.memset(onesf[:], 1.0)
            ones = consts.tile([1, BP], BF16, tag="ones")
            nc.vector.tensor_copy(ones[:], onesf[:])
            zb16 = consts.tile([128, 4644], BF16, tag="zb16")
            nc.vector.memset(zb16[:], 0.0)

            # ---------------- style maps: all 16 samples ----------------
            wsb = consts.tile([128, 4, B, 9], F32, tag="wsb")
            for q in range(4):
                nc.sync.dma_start(
                    out=wsb[:, q, :, :],
                    in_=wall.rearrange("b s kh kw -> s b (kh kw)")[
                        128 * q : 128 * (q + 1), :, :
                    ],
                )
            # reflect-pad 3x3 -> 5x5 (batched over b), bf16
            wp = consts.tile([128, 4, B, 5, 5], BF16, tag="wp")
            for q in range(4):
                w3 = wsb[:, q, :, :].rearrange("p b (kh kw) -> p b kh kw", kh=3)
                nc.vector.tensor_copy(wp[:, q, :, 1:4, 1:4], w3)
                nc.vector.tensor_copy(wp[:, q, :, 1:4, 0:1], w3[:, :, :, 1:2])
                nc.vector.tensor_copy(wp[:, q, :, 1:4, 4:5], w3[:, :, :, 1:2])
                nc.vector.tensor_copy(wp[:, q, :, 0, :], wp[:, q, :, 2, :])
                nc.vector.tensor_copy(wp[:, q, :, 4, :], wp[:, q, :, 2, :])

            # im2col of padded style map: [s-chunk, kpos, q, (b, opos)]
            xw = consts.tile([128, 9, 4, BP], BF16, tag="xw")
            for di in range(3):
                for dj in range(3):
                    for q in range(4):
                        nc.vector.tensor_copy(
                            xw[:, di * 3 + dj, q, :].rearrange(
                                "p (b i j) -> p b i j", b=B, i=3
                            ),
                            wp[:, q, :, di : di + 3, dj : dj + 3],
                        )

            pooled_f = consts.tile([128, 4, B], F32, tag="pooledf")
            for q in range(4):
                nc.vector.tensor_reduce(
                    out=pooled_f[:, q, :],
                    in_=wsb[:, q, :, :],
                    axis=mybir.AxisListType.X,
                    op=AluOpType.add,
                )
            pooledb = consts.tile([128, 4, B], BF16, tag="pooledb")
            nc.scalar.mul(pooledb[:, :, :], pooled_f[:, :, :], 1.0 / 9.0)

            # own-sample pooled style (for the local bias predictor)
            wosb = consts.tile([128, 4, BPC, 9], F32, tag="wosb")
            for q in range(4):
                nc.sync.dma_start(
                    out=wosb[:, q, :, :],
                    in_=wown.rearrange("b s kh kw -> s b (kh kw)")[
                        128 * q : 128 * (q + 1), :, :
                    ],
                )
            pooledo_f = consts.tile([128, 4, BPC], F32, tag="pooledof")
            for q in range(4):
                nc.vector.tensor_reduce(
                    out=pooledo_f[:, q, :],
                    in_=wosb[:, q, :, :],
                    axis=mybir.AxisListType.X,
                    op=AluOpType.add,
                )
            pooledo = consts.tile([128, 4, BPC], BF16, tag="pooledo")
            nc.scalar.mul(pooledo[:, :, :], pooledo_f[:, :, :], 1.0 / 9.0)

            # ------- predictor: dw cols (this core's 512-col slice) -------
            dwTs = consts.tile([128, 4, B, NTAP], BF16, tag="dwTs")
            pss = [psum.tile([128, 512], F32, tag="mm", name=f"pp{i}") for i in range(4)]
            for kpos in range(9):
                ksb = kstream.tile([128, 4, 512], BF16, tag="ksb")
                nc.sync.dma_start(
                    out=ksb[:],
                    in_=kpsws[kpos * 512 : (kpos + 1) * 512, :].rearrange(
                        "(q s) c -> s q c", q=4
                    ),
                )
                for q in range(4):
                    for ct in range(4):
                        nc.tensor.matmul(
                            pss[ct][:, :BP],
                            ksb[:, q, 128 * ct : 128 * (ct + 1)],
                            xw[:, kpos, q, :],
                            start=(kpos == 0 and q == 0),
                            stop=False,
                        )
            rb = consts.tile([1, 512], BF16, tag="rb")
            nc.sync.dma_start(out=rb[:], in_=kpsws[9 * S : 9 * S + 1, :])
            for ct in range(4):
                nc.tensor.matmul(
                    pss[ct][:, :BP],
                    rb[:1, 128 * ct : 128 * (ct + 1)],
                    ones[:1, :BP],
                    start=False,
                    stop=True,
                )
            for ct in range(4):
                nc.vector.tensor_copy(
                    dwTs[:, ct, :, :9],
                    pss[ct][:, :BP].rearrange("p (b pos) -> p b pos", b=B),
                )

            # ------- predictor: pw cols -> tap 9 of dwTs -------
            kpsb = consts.tile([128, 4, 512], BF16, tag="kpsb")
            nc.sync.dma_start(
                out=kpsb[:], in_=kppws[:S, :].rearrange("(q s) c -> s q c", q=4)
            )
            rbp = consts.tile([1, 512], BF16, tag="rbp")
            nc.sync.dma_start(out=rbp[:], in_=kppws[S : S + 1, :])
            for ct in range(4):
                ps2 = psum.tile([128, 512], F32, tag="mm")
                for q in range(4):
                    nc.tensor.matmul(
                        ps2[:, :B],
                        kpsb[:, q, 128 * ct : 128 * (ct + 1)],
                        pooledb[:, q, :],
                        start=(q == 0),
                        stop=False,
                    )
                nc.tensor.matmul(
                    ps2[:, :B],
                    rbp[:1, 128 * ct : 128 * (ct + 1)],
                    ones[:1, :B],
                    start=False,
                    stop=True,
                )
                nc.vector.tensor_copy(dwTs[:, ct, :, 9], ps2[:, :B])

            # ------- pack cc input + AllToAll -------
            # slab layout per sample: k = c_local*NTAP + tap, tap 0..8 = dw
            # output positions, tap 9 = pw (tap-minor so both DMA sides end
            # on a contiguous dim).
            for ct in range(4):
                nc.sync.dma_start(
                    out=ccin.rearrange("b (c pos) -> c b pos", pos=NTAP)[
                        128 * ct : 128 * (ct + 1)
                    ],
                    in_=dwTs[:, ct, :, :],
                )
            nc.gpsimd.collective_compute(
                "AllToAll",
                AluOpType.bypass,
                replica_groups=[list(range(NCORES))],
                ins=[ccin.ap().opt()],
                outs=[ccout.ap().opt()],
            )

            # ------- local bias predictor (own 2 samples, overlaps the cc) ---
            kbsb = consts.tile([128, 4, C], BF16, tag="kbsb")
            nc.sync.dma_start(
                out=kbsb[:], in_=kpbw[:S, :].rearrange("(q s) c -> s q c", q=4)
            )
            rbb = consts.tile([1, C], BF16, tag="rbb")
            nc.sync.dma_start(out=rbb[:], in_=kpbw[S : S + 1, :])
            biasc = consts.tile([128, NT, BPC], F32, tag="biasc")
            for m in range(NT):
                ps3 = psum.tile([128, 512], F32, tag="mm")
                for q in range(4):
                    nc.tensor.matmul(
                        ps3[:, :BPC],
                        kbsb[:, q, 128 * m : 128 * (m + 1)],
                        pooledo[:, q, :],
                        start=(q == 0),
                        stop=False,
                    )
                nc.tensor.matmul(
                    ps3[:, :BPC],
                    rbb[:1, 128 * m : 128 * (m + 1)],
                    ones[:1, :BPC],
                    start=False,
                    stop=True,
                )
                nc.vector.tensor_copy(biasc[:, m, :], ps3[:, :BPC])

            # ------- overlapped with cc: identity matrix, weights, x-norm ---
            # Build idT[i, g, m] = (m == 8g+i) via a DRAM stride-129 diagonal.
            onescol = consts.tile([128, 1], BF16, tag="onescol")
            nc.vector.memset(onescol[:], 1.0)
            nc.sync.dma_start(
                out=idscr.rearrange("(p c) -> p c", c=129), in_=zb16[:, :129]
            )
            nc.sync.dma_start(
                out=idscr.rearrange("(p c) -> p c", c=129)[:, :1], in_=onescol[:]
            )
            idT = consts.tile([8, 16, 128], BF16, tag="idT")
            nc.sync.dma_start(
                out=idT[:],
                in_=idscr[: 128 * 128]
                .rearrange("(gi m) -> gi m", m=128)
                .rearrange("(g i) m -> i g m", g=16),
            )

            # decoder conv weights, SBUF-resident for the whole kernel
            w1sb = []
            for k in range(NT):
                t_ = wres.tile([128, 9, C], BF16, tag=f"w1k{k}")
                w1sb.append(t_)
                nc.sync.dma_start(
                    out=t_[:],
                    in_=w1t[:, 128 * k : 128 * (k + 1), :].rearrange(
                        "pos p co -> p pos co"
                    ),
                )
            w2sb = []
            for k in range(NT):
                t_ = wres.tile([128, 9, CO], BF16, tag=f"w2k{k}")
                w2sb.append(t_)
                nc.sync.dma_start(
                    out=t_[:],
                    in_=w2t[:, 128 * k : 128 * (k + 1), :].rearrange(
                        "pos p co -> p pos co"
                    ),
                )

            # instance norm -> reflect-padded xn (bf16)
            xps = []
            for b in range(BPC):
                xp = pad3.tile([128, NT, 34, 34], BF16, tag="padbuf")
                xps.append(xp)
                for t in range(NT):
                    xsb = actp.tile([128, HW], F32, tag="xsb")
                    nc.sync.dma_start(
                        out=xsb[:],
                        in_=x2[b, 128 * t : 128 * (t + 1), :, :].rearrange(
                            "c h w -> c (h w)"
                        ),
                    )
                    st = actp.tile([128, 2, 6], F32, tag="bnst")
                    xsb2 = xsb[:].rearrange("p (s f) -> p s f", f=512)
                    for sg in range(2):
                        nc.vector.bn_stats(out=st[:, sg, :], in_=xsb2[:, sg, :])
                    mv = actp.tile([128, 2], F32, tag="bnmv")
                    nc.vector.bn_aggr(out=mv[:], in_=st[:])
                    rstd = actp.tile([128, 1], F32, tag="rstd")
                    nc.scalar.activation(
                        out=rstd[:], in_=mv[:, 1:2], func=AF.Sqrt, bias=epsb[:], scale=1.0
                    )
                    nc.vector.reciprocal(out=rstd[:], in_=rstd[:])
                    nc.vector.tensor_scalar(
                        out=xp[:, t, 1:33, 1:33],
                        in0=xsb[:].rearrange("p (h w) -> p h w", h=H),
                        scalar1=mv[:, 0:1],
                        scalar2=rstd[:],
                        op0=AluOpType.subtract,
                        op1=AluOpType.mult,
                    )
                    nc.vector.tensor_copy(xp[:, t, 1:33, 0:1], xp[:, t, 1:33, 2:3])
                    nc.vector.tensor_copy(xp[:, t, 1:33, 33:34], xp[:, t, 1:33, 31:32])
                    nc.vector.tensor_copy(xp[:, t, 0, :], xp[:, t, 2, :])
                    nc.vector.tensor_copy(xp[:, t, 33, :], xp[:, t, 31, :])

            # zero-padded output buffers for the decoder convs
            yp1s, yp2s = [], []
            for b in range(BPC):
                yp1 = pad3.tile([128, NT, 34, 34], BF16, tag="padbuf")
                yp1s.append(yp1)
                yp2 = pad3.tile([128, NT, 34, 34], BF16, tag="padbuf")
                yp2s.append(yp2)
                for yp in (yp1, yp2):
                    for t in range(NT):
                        nc.vector.tensor_copy(yp[:, t, 0, :], zb16[:, :34])
                        nc.vector.tensor_copy(yp[:, t, 33, :], zb16[:, :34])
                        nc.vector.tensor_copy(
                            yp[:, t, 1:33, 0:1],
                            zb16[:, :32].rearrange("p (a c) -> p a c", c=1),
                        )
                        nc.vector.tensor_copy(
                            yp[:, t, 1:33, 33:34],
                            zb16[:, :32].rearrange("p (a c) -> p a c", c=1),
                        )

            # ---------------- phase A: adaconv (dynamic grouped conv) --------
            # dsrcdw[i, gh, gl, (co pos)] / dsrcpw[i, gh, gl, co] <- ccout.
            # Expand to block-diag [128,128] tiles with identity-selector
            # matmuls: out[p, c] = sum_i idT[i, g, c] rhs[i, n] puts each
            # group's 8x8 block at rows/cols 8g and zeros elsewhere.
            POSG = ((0, 4), (4, 4), (8, 1))  # dw psum passes over the 9 taps
            for b in range(BPC):
                xp = xps[b]
                yp1 = yp1s[b]
                for t in range(NT):
                    dsrcdw = blkp.tile([8, 2, 8, 72], BF16, tag="dsrcdw")
                    dsrcpw = blkp.tile([8, 2, 8, 8], BF16, tag="dsrcpw")
                    for gh in range(2):
                        row = 2 * (2 * t + gh) + b
                        nc.sync.dma_start(
                            out=dsrcdw[:, gh, :, :],
                            in_=ccout[row, : 9 * CSL].rearrange(
                                "(gl i co pos) -> i gl (co pos)", gl=8, i=8, co=8
                            ),
                        )
                        nc.sync.dma_start(
                            out=dsrcpw[:, gh, :, :],
                            in_=ccout[row, 9 * CSL :].rearrange(
                                "(gl i co) -> i gl co", gl=8, i=8
                            ),
                        )
                    dwpw = blkp.tile([128, 10, 128], BF16, tag="dwpw")
                    for p0, np_ in POSG:
                        psd = psum.tile([128, 512], F32, tag="mm")
                        for g in range(16):
                            gh, gl = g // 8, g % 8
                            nc.tensor.matmul(
                                psd[:, : np_ * 128].rearrange(
                                    "p (pos c) -> p pos c", pos=np_
                                )[:, :, 8 * g : 8 * (g + 1)],
                                idT[:, g, :],
                                dsrcdw[:, gh, gl, :].rearrange(
                                    "i (co pos) -> i pos co", co=8
                                )[:, p0 : p0 + np_, :],
                                start=True,
                                stop=True,
                            )
                        if p0 == 8:  # fold the pw tile into the same pass
                            for g in range(16):
                                gh, gl = g // 8, g % 8
                                nc.tensor.matmul(
                                    psd[:, 128 + 8 * g : 128 + 8 * (g + 1)],
                                    idT[:, g, :],
                                    dsrcpw[:, gh, gl, :],
                                    start=True,
                                    stop=True,
                                )
                            nc.vector.tensor_copy(
                                dwpw[:, 8:10, :],
                                psd[:, :256].rearrange("p (pos c) -> p pos c", pos=2),
                            )
                        else:
                            nc.vector.tensor_copy(
                                dwpw[:, p0 : p0 + np_, :],
                                psd[:, : np_ * 128].rearrange(
                                    "p (pos c) -> p pos c", pos=np_
                                ),
                            )
                    ysb = actp.tile([128, HW], BF16, tag="ysb")
                    for hh in range(2):
                        ps = psum.tile([128, 512], F32, tag="mm")
                        for kdi in range(3):
                            for kdj in range(3):
                                pos = kdi * 3 + kdj
                                nc.tensor.matmul(
                                    ps[:],
                                    dwpw[:, pos, :],
                                    xp[:, t, kdi + 16 * hh : kdi + 16 * hh + 16, kdj : kdj + 32],
                                    start=(pos == 0),
                                    stop=(pos == 8),
                                )
                        nc.vector.tensor_copy(ysb[:, 512 * hh : 512 * (hh + 1)], ps[:])
                    for hh in range(2):
                        ps2 = psum.tile([128, 512], F32, tag="mm")
                        nc.tensor.matmul(
                            ps2[:],
                            dwpw[:, 9, :],
                            ysb[:, 512 * hh : 512 * (hh + 1)],
                            start=True,
                            stop=True,
                        )
                        nc.scalar.activation(
                            out=yp1[:, t, 1 + 16 * hh : 17 + 16 * hh, 1:33],
                            in_=ps2[:].rearrange("p (h w) -> p h w", h=16),
                            func=AF.Identity,
                            bias=biasc[:, t, b : b + 1],
                            scale=1.0,
                        )

            # ---------------- phase B: conv1 (512 -> 512) + relu ----------------
            for m in range(NT):
                pss2 = [
                    psum.tile([128, 512], F32, tag="mm", name=f"pb{i}")
                    for i in range(2 * BPC)
                ]
                for k in range(NT):
                    for b in range(BPC):
                        for hh in range(2):
                            ps = pss2[2 * b + hh]
                            for kdi in range(3):
                                for kdj in range(3):
                                    pos = kdi * 3 + kdj
                                    nc.tensor.matmul(
                                        ps[:],
                                        w1sb[k][:, pos, 128 * m : 128 * (m + 1)],
                                        yp1s[b][:, k, kdi + 16 * hh : kdi + 16 * hh + 16, kdj : kdj + 32],
                                        start=(k == 0 and pos == 0),
                                        stop=(k == NT - 1 and pos == 8),
                                    )
                for b in range(BPC):
                    for hh in range(2):
                        nc.scalar.activation(
                            out=yp2s[b][:, m, 1 + 16 * hh : 17 + 16 * hh, 1:33],
                            in_=pss2[2 * b + hh][:].rearrange("p (h w) -> p h w", h=16),
                            func=AF.Relu,
                            bias=b1sb[:, m : m + 1],
                            scale=1.0,
                        )

            # ------- phase C: conv2 (512 -> 256) + relu + 2x upsample -------
            for m2 in range(NM2):
                pss2 = [
                    psum.tile([128, 512], F32, tag="mm", name=f"pc{i}")
                    for i in range(2 * BPC)
                ]
                for k in range(NT):
                    for b in range(BPC):
                        for hh in range(2):
                            ps = pss2[2 * b + hh]
                            for kdi in range(3):
                                for kdj in range(3):
                                    pos = kdi * 3 + kdj
                                    nc.tensor.matmul(
                                        ps[:],
                                        w2sb[k][:, pos, 128 * m2 : 128 * (m2 + 1)],
                                        yp2s[b][:, k, kdi + 16 * hh : kdi + 16 * hh + 16, kdj : kdj + 32],
                                        start=(k == 0 and pos == 0),
                                        stop=(k == NT - 1 and pos == 8),
                                    )
                for b in range(BPC):
                    for hh in range(2):
                        ps = pss2[2 * b + hh]
                        # ous[p, h, two, w'] with w' = 2w+dup; rows duplicated
                        # via `two`, cols via `dup` -> 8KB contiguous DRAM runs
                        ous = outp.tile([128, 16, 2, 64], F32, tag="ous")
                        for two in range(2):
                            for dup in range(2):
                                nc.scalar.activation(
                                    out=ous[:, :, two, :].rearrange(
                                        "p h (w dup) -> p h dup w", dup=2
                                    )[:, :, dup, :],
                                    in_=ps[:].rearrange("p (h w) -> p h w", h=16),
                                    func=AF.Relu,
                                    bias=b2sb[:, m2 : m2 + 1],
                                    scale=1.0,
                                )
                        nc.sync.dma_start(
                            out=yout[b, 128 * m2 : 128 * (m2 + 1), :, :]
                            .rearrange("c (blk h2) w -> c blk h2 w", blk=2)[:, hh]
                            .rearrange("c (h two) w -> c h two w", two=2),
                            in_=ous[:],
                        )

    nc.compile()
    return nc


def _repack(inputs):
    bf = ml_dtypes.bfloat16
    kp_sw = np.ascontiguousarray(inputs["kp_sw"], dtype=np.float32)
    kp_sb = np.ascontiguousarray(inputs["kp_sb"], dtype=np.float32)
    kp_pw = np.ascontiguousarray(inputs["kp_pw"], dtype=np.float32)
    kp_pb = np.ascontiguousarray(inputs["kp_pb"], dtype=np.float32)
    kp_bw = np.ascontiguousarray(inputs["kp_bw"], dtype=np.float32)
    kp_bb = np.ascontiguousarray(inputs["kp_bb"], dtype=np.float32)
    dec_w1 = np.ascontiguousarray(inputs["dec_w1"], dtype=np.float32)
    dec_b1 = np.ascontiguousarray(inputs["dec_b1"], dtype=np.float32)
    dec_w2 = np.ascontiguousarray(inputs["dec_w2"], dtype=np.float32)
    dec_b2 = np.ascontiguousarray(inputs["dec_b2"], dtype=np.float32)

    # column permutation: position (t, g, i, co) <- original o = (c_out, i)
    O = np.arange(C * CG).reshape(NT, 16, CG, CG)  # (t, g, co, i), o-major
    P = O.transpose(0, 1, 3, 2).reshape(-1)        # (t, g, i, co)

    kpsw = np.empty((9 * S + 1, C * CG), dtype=np.float32)
    kpsw[: 9 * S] = (
        kp_sw[P].reshape(C * CG, S, 3, 3).transpose(2, 3, 1, 0).reshape(9 * S, C * CG)
    )  # rows in k-order (di, dj, s)
    kpsw[9 * S] = kp_sb[P]

    kppw = np.empty((S + 1, C * CG), dtype=np.float32)
    kppw[:S] = kp_pw[P].T
    kppw[S] = kp_pb[P]

    kpbw = np.empty((S + 1, C), dtype=np.float32)
    kpbw[:S] = kp_bw.T
    kpbw[S] = kp_bb

    w1 = np.ascontiguousarray(dec_w1.transpose(2, 3, 1, 0).reshape(9, C, C))
    w2 = np.ascontiguousarray(dec_w2.transpose(2, 3, 1, 0).reshape(9, C, CO))

    shared = {
        "w1t": w1.astype(bf),
        "w2t": w2.astype(bf),
        "b1d": dec_b1,
        "b2d": dec_b2,
        "kpbw": kpbw.astype(bf),
    }
    slices = []
    for c in range(NCORES):
        slices.append(
            {
                "kpsws": np.ascontiguousarray(
                    kpsw[:, CSL * c : CSL * (c + 1)]
                ).astype(bf),
                "kppws": np.ascontiguousarray(
                    kppw[:, CSL * c : CSL * (c + 1)]
                ).astype(bf),
            }
        )
    return shared, slices


def kernel(**inputs):
    if "nc" not in _CACHE:
        _CACHE["nc"] = _build()
    nc = _CACHE["nc"]

    shared, slices = _repack(inputs)
    x = np.ascontiguousarray(inputs["x"], dtype=np.float32)
    w = np.ascontiguousarray(inputs["w"], dtype=np.float32)

    in_maps = []
    for c in range(NCORES):
        sl = slice(BPC * c, BPC * (c + 1))
        in_maps.append(
            {"x2": x[sl], "wall": w, "wown": w[sl], **shared, **slices[c]}
        )

    res = run_bass_kernel_spmd(nc, in_maps, list(range(NCORES))).results
    return np.concatenate([r["yout"] for r in res], axis=0)
